# revision 36
# baseline (speedup 1.0000x reference)
"""Trainium2 Bass kernel for the interaction-network GNN (nn_Physics_7370163880185).

Reference computation (per batch element b, objects i=0..15, D=256):
  trans  = MLP_t(objs)                              # 256->512->512->256, relu x3
  pair(i,j) = concat(objs_i, objs_j)                # [512]
  inter  = MLP_i(pair)                              # 512->512->512->256, relu x3
  out    = trans + sum_{j != i} inter(i,j) + objs

Sharding: data-parallel over batch B=512 across 8 cores (64 per core).

Kernel strategy (per core):
  * Feature-on-partition layout: activations are [feat, rows] with
    rows = (n, b) flattened; matmul(out, lhsT=W[k,m], rhs=xT[k, rows])
    chains layers without transposes.
  * Interaction layer 1 is split: concat(a,b) @ iW1 = a @ iW1[:256] + b @ iW1[256:],
    so U = objs @ iW1[:256] + ib1 and V = objs @ iW1[256:] are computed once on
    N*B rows; h1(i,j) = relu(U_i + V_j) is a broadcast add.
  * All matmuls run in bf16 (inputs rounded host-side; fp32 PSUM accumulate).
  * Phase 2 iterates over rotations s=1..15: rotation s computes
    inter(i, (i+s) mod 16) for all i at once in (i, b) column order, so the
    masked diagonal is never computed, h1 = relu(U + roll(V, s)) is two
    dense shifted adds (no broadcast), and the j-sum is a running
    accumulation acc += relu(L3 + ib3) (no masking, no tree reduce).
  * Engine balance: DVE does the h1 adds (bf16 2x) + in-place relu (bf16
    4x); the scalar engine drains all PSUM outputs; gpsimd owns the acc
    accumulation; h1 for s=1,2 is emitted between phase-1 layers so the
    DVE reaches it before the t-MLP finishes on the PE.
  * Weights/biases are packed host-side into per-tensor [128, nk*fout]
    layouts; DMAs stay per-k-tile (parallel rings beat one big serial DMA).
"""

import numpy as np

import concourse.bass as bass
import concourse.mybir as mybir
import concourse.tile as tile
from concourse import bacc
from concourse.bass_utils import run_bass_kernel_spmd

N = 16
B = 512
D = 256
NCORES = 8
BL = B // NCORES          # 64 batch rows per core
ROWS = N * BL             # 1024 (n, b) rows per core
PT = 128                  # partition tile
NT = 512                  # matmul free-dim tile (one PSUM bank of fp32)

F32 = mybir.dt.float32
BF16 = mybir.dt.bfloat16
RELU = mybir.ActivationFunctionType.Relu
IDENT = mybir.ActivationFunctionType.Identity
ADD = mybir.AluOpType.add
MAX = mybir.AluOpType.max

# weight tensors, packed [128, nk*fout] host-side (k-tiles side by side)
WEIGHT_SHAPES = {
    "tW1": (256, 512), "tW2": (512, 512), "tW3": (512, 256),
    "iW1": (256, 512), "iW1b": (256, 512),
    "iW2": (512, 512), "iW3": (512, 256),
}
# biases packed into one [128, 20] tensor, in this column order
BIAS_COLS = {"tb1": (0, 4), "tb2": (4, 8), "tb3": (8, 10),
             "ib1": (10, 14), "ib2": (14, 18), "ib3": (18, 20)}


def _build_body(nc, tc, prm, ctx):
    cpool = ctx.enter_context(tc.tile_pool(name="const", bufs=1))
    wpool = ctx.enter_context(tc.tile_pool(name="work", bufs=2))
    ppool = ctx.enter_context(tc.tile_pool(name="psum", bufs=2, space="PSUM"))

    # ---- inputs: packed SBUF layout, per-k-tile DMAs (parallel rings),
    # in first-use order ---------------------------------------------------
    objsb = cpool.tile([PT, 2 * ROWS], BF16, tag="objsb", bufs=1, name="objsb")
    for p in range(2):
        nc.sync.dma_start(out=objsb[:, p * ROWS:(p + 1) * ROWS],
                          in_=prm["objsb"][:, p * ROWS:(p + 1) * ROWS])
    objsT_b = [objsb[:, p * ROWS:(p + 1) * ROWS] for p in range(2)]

    w_sb = {}

    def load_weights(*names):
        for wname in names:
            fin, fout = WEIGHT_SHAPES[wname]
            nk = fin // PT
            t = cpool.tile([PT, nk * fout], BF16, tag=wname, bufs=1, name=wname)
            for k in range(nk):
                nc.sync.dma_start(out=t[:, k * fout:(k + 1) * fout],
                                  in_=prm[wname][:, k * fout:(k + 1) * fout])
            w_sb[wname] = t

    # only what phase 1's start needs; the rest is DMA'd mid-phase-1 so the
    # first matmuls don't sit behind the whole input queue
    load_weights("iW1", "iW1b", "tW1", "tW2", "tW3", "iW2", "iW3")

    def wslice(wname, k, m):
        fout = WEIGHT_SHAPES[wname][1]
        return w_sb[wname][:, k * fout + m * PT:k * fout + (m + 1) * PT]

    biases = cpool.tile([PT, 20], F32, tag="biases", bufs=1, name="biases")
    nc.sync.dma_start(out=biases, in_=prm["biases"][:, :])

    def bias(bname, m):
        lo, hi = BIAS_COLS[bname]
        assert lo + m < hi
        return biases[:, lo + m:lo + m + 1]

    objsf = cpool.tile([PT, 2 * ROWS], F32, tag="objsf", bufs=1, name="objsf")
    for p in range(2):
        nc.sync.dma_start(out=objsf[:, p * ROWS:(p + 1) * ROWS],
                          in_=prm["objs"][:, p * ROWS:(p + 1) * ROWS])
    objsT_f = [objsf[:, p * ROWS:(p + 1) * ROWS] for p in range(2)]

    # ---- generic dense layer over the full ROWS ------------------------
    # Pairs of column-chunks share each 128-col weight tile so LDWEIGHTS
    # amortizes over 2 matmuls, and consecutive MMs hit different banks.
    def layer(wname, rhs, drain):
        fin, fout = WEIGHT_SHAPES[wname]
        nk = fin // PT
        nm = fout // PT
        ncols = rhs[0].shape[-1]
        mc = [(m, c) for m in range(nm) for c in range(ncols // NT)]
        for g in range(0, len(mc), 2):
            grp = mc[g:g + 2]
            pss = [ppool.tile([PT, NT], F32, tag="ps", bufs=8,
                              name=f"ps_{wname}_{m}_{c}") for (m, c) in grp]
            for k in range(nk):
                for (m, c), ps in zip(grp, pss):
                    nc.tensor.matmul(
                        ps, wslice(wname, k, m),
                        rhs[k][:, c * NT:(c + 1) * NT],
                        start=(k == 0), stop=(k == nk - 1))
            for (m, c), ps in zip(grp, pss):
                drain(m, c, ps)

    def persist(tag, n_tiles, dt=BF16, cols=ROWS):
        return [cpool.tile([PT, cols], dt, tag=f"{tag}_{m}", bufs=1, name=f"{tag}_{m}")
                for m in range(n_tiles)]

    # ---- phase 1: U, V, trans MLP (rows = (n, b), 1024) -----------------
    U = persist("U", 4)       # bf16(objs @ iW1[:256] + ib1)
    V = persist("V", 4)       # bf16(objs @ iW1[256:])
    t1r = persist("t1r", 4)
    t2r = persist("t2r", 4)
    t3 = persist("t3", 2, F32)   # becomes S = t3 + objs after in-place add

    # h1 for rotation s: h1 = relu(U + roll(V, s)) — two dense shifted adds
    # plus an in-place relu, all on DVE (bf16 2x / 4x modes).
    def make_h1(s):
        sp = (N - s) * BL     # split point: i < N-s reads V at +s*BL
        h1r = [wpool.tile([PT, ROWS], BF16, tag=f"h1r_{p}", bufs=3,
                          name=f"h1r_{s}_{p}") for p in range(4)]
        for p in range(4):
            nc.vector.tensor_add(h1r[p][:, 0:sp], U[p][:, 0:sp],
                                 V[p][:, s * BL:ROWS])
            nc.vector.tensor_add(h1r[p][:, sp:ROWS], U[p][:, sp:ROWS],
                                 V[p][:, 0:s * BL])
            nc.vector.tensor_scalar(h1r[p], h1r[p], 0.0, None, MAX)
        return h1r

    # Phase-1 drains stay off the DVE (scalar + gpsimd) so the DVE queue
    # reaches the first rotations' h1 work while the PE runs the t-MLP.
    h1_pre = {}
    layer("iW1", objsT_b,
          lambda m, c, ps: nc.scalar.activation(
              U[m][:, c * NT:(c + 1) * NT], ps, IDENT, bias=bias("ib1", m)))
    layer("iW1b", objsT_b,
          lambda m, c, ps: nc.vector.tensor_copy(V[m][:, c * NT:(c + 1) * NT], ps))
    h1_pre[1] = make_h1(1)
    layer("tW1", objsT_b,
          lambda m, c, ps: nc.scalar.activation(
              t1r[m][:, c * NT:(c + 1) * NT], ps, RELU, bias=bias("tb1", m)))
    h1_pre[2] = make_h1(2)
    layer("tW2", t1r,
          lambda m, c, ps: nc.scalar.activation(
              t2r[m][:, c * NT:(c + 1) * NT], ps, RELU, bias=bias("tb2", m)))
    layer("tW3", t2r,
          lambda m, c, ps: nc.scalar.activation(
              t3[m][:, c * NT:(c + 1) * NT], ps, RELU, bias=bias("tb3", m)))
    # S = t3 + objs  (in place, gpsimd: DVE is on the h1 chain)
    for p in range(2):
        nc.gpsimd.tensor_add(t3[p], t3[p], objsT_f[p])

    # ---- phase 2: rotation loop ----------------------------------------
    # Rotation s pairs every i with j = (i+s) mod 16 at once, skipping the
    # masked diagonal entirely. Column order everywhere is (i, b), matching
    # U, t3 and the output, so the j-sum becomes a running accumulation
    # acc += relu(L3 + ib3) with no masking or tree.
    acc = [cpool.tile([PT, ROWS], F32, tag=f"acc_{p}", bufs=1, name=f"acc_{p}")
           for p in range(2)]
    for s in range(1, N):
        h1r = h1_pre.pop(s) if s in h1_pre else make_h1(s)

        # L2: groups of (m, ih-pair): 2 banks per group, each 128-col weight
        # feeds 2 consecutive MMs (ih inner), 4 groups pipeline in 8 banks
        xT2 = [[wpool.tile([PT, NT], BF16, tag=f"xT2_{ih}_{m}", bufs=2,
                           name=f"xT2_{s}_{ih}_{m}") for m in range(4)]
               for ih in range(2)]
        for m in range(4):
            pss = [ppool.tile([PT, NT], F32, tag="ps", bufs=8,
                              name=f"psL2_{s}_{ih}_{m}") for ih in range(2)]
            for k in range(4):
                for ih in range(2):
                    nc.tensor.matmul(pss[ih], wslice("iW2", k, m),
                                     h1r[k][:, ih * NT:(ih + 1) * NT],
                                     start=(k == 0), stop=(k == 3))
            for ih in range(2):
                nc.scalar.activation(xT2[ih][m], pss[ih], RELU,
                                     bias=bias("ib2", m))
        # L3 + accumulate: s==1 drains straight into acc; later rotations
        # drain to a temp (scalar) and gpsimd adds it into acc.
        for m in range(2):
            pss3 = [ppool.tile([PT, NT], F32, tag="ps", bufs=8,
                               name=f"psL3_{s}_{ih}_{m}") for ih in range(2)]
            for k in range(4):
                for ih in range(2):
                    nc.tensor.matmul(pss3[ih], wslice("iW3", k, m),
                                     xT2[ih][k], start=(k == 0), stop=(k == 3))
            for ih in range(2):
                cs = slice(ih * NT, (ih + 1) * NT)
                if s == 1:
                    nc.scalar.activation(acc[m][:, cs], pss3[ih], RELU,
                                         bias=bias("ib3", m))
                elif s < N - 1:
                    tmp = wpool.tile([PT, NT], F32, tag=f"tmp_{ih}_{m}",
                                     bufs=2, name=f"tmp_{s}_{ih}_{m}")
                    nc.scalar.activation(tmp, pss3[ih], RELU,
                                         bias=bias("ib3", m))
                    nc.gpsimd.tensor_add(acc[m][:, cs], acc[m][:, cs], tmp)
                else:
                    # last rotation: finish this (m, ih) quarter end-to-end
                    # (acc-add + residual + store) so the tail pipelines
                    tmp = wpool.tile([PT, NT], F32, tag=f"tmp_{ih}_{m}",
                                     bufs=2, name=f"tmp_{s}_{ih}_{m}")
                    nc.scalar.activation(tmp, pss3[ih], RELU,
                                         bias=bias("ib3", m))
                    nc.vector.tensor_add(tmp, acc[m][:, cs], tmp)
                    osb = wpool.tile([PT, NT], F32, tag=f"osb_{ih}_{m}",
                                     bufs=1, name=f"osb_{ih}_{m}")
                    nc.vector.tensor_add(osb, tmp, t3[m][:, cs])
                    nc.sync.dma_start(
                        out=prm["out"].rearrange(
                            "(h p) n b -> p h n b", h=2)[:, m, ih * 8:(ih + 1) * 8],
                        in_=osb.rearrange("p (n b) -> p n b", n=8))


def build_nc(loop_iters=None):
    """loop_iters: if set, wrap the whole body in a hardware For_i loop that
    repeats it that many times (used only for timing measurements)."""
    nc = bacc.Bacc("TRN2", target_bir_lowering=False, debug=False)
    prm = {}
    prm["objs"] = nc.declare_dram_parameter("objs", [PT, 2 * ROWS], F32, isOutput=False)
    prm["objsb"] = nc.declare_dram_parameter("objsb", [PT, 2 * ROWS], BF16, isOutput=False)
    for wname, (fin, fout) in WEIGHT_SHAPES.items():
        nk = fin // PT
        prm[wname] = nc.declare_dram_parameter(wname, [PT, nk * fout], BF16, isOutput=False)
    prm["biases"] = nc.declare_dram_parameter("biases", [PT, 20], F32, isOutput=False)
    prm["out"] = nc.declare_dram_parameter("out", [D, N, BL], F32, isOutput=True)
    from contextlib import ExitStack
    with tile.TileContext(nc) as tc:
        if loop_iters is None:
            with ExitStack() as ctx:
                _build_body(nc, tc, prm, ctx)
        else:
            with tc.For_i(0, loop_iters, 1):
                with ExitStack() as ctx:
                    _build_body(nc, tc, prm, ctx)
    nc.compile()
    return nc


_CACHE = {}


def _get_nc():
    if "nc" not in _CACHE:
        _CACHE["nc"] = build_nc()
    return _CACHE["nc"]


def _pack_ktiles(w):
    """[fin, fout] -> [128, nk*fout] with k-tiles side by side."""
    fin, fout = w.shape
    nk = fin // PT
    return np.ascontiguousarray(
        w.reshape(nk, PT, fout).transpose(1, 0, 2).reshape(PT, nk * fout))


def make_in_maps(inputs):
    import ml_dtypes
    BF = ml_dtypes.bfloat16
    shared = {}
    for name in ("tW1", "tW2", "tW3", "iW2", "iW3"):
        shared[name] = _pack_ktiles(
            np.asarray(inputs[name], dtype=np.float32)).astype(BF)
    iW1 = np.asarray(inputs["iW1"], dtype=np.float32)
    shared["iW1"] = _pack_ktiles(iW1[:D]).astype(BF)
    shared["iW1b"] = _pack_ktiles(iW1[D:]).astype(BF)
    bcols = []
    for bname in BIAS_COLS:
        b = np.asarray(inputs[bname], dtype=np.float32)
        bcols.append(b.reshape(-1, PT).T)       # [128, nb]
    shared["biases"] = np.ascontiguousarray(np.concatenate(bcols, axis=1))
    objs = np.asarray(inputs["objs"], dtype=np.float32)
    in_maps = []
    for c in range(NCORES):
        m = dict(shared)
        sl = objs[:, c * BL:(c + 1) * BL, :]            # [N, BL, D]
        oT = sl.transpose(2, 0, 1).reshape(D, ROWS)     # [256, 1024]
        oP = np.ascontiguousarray(
            oT.reshape(2, PT, ROWS).transpose(1, 0, 2).reshape(PT, 2 * ROWS))
        m["objs"] = oP
        m["objsb"] = np.ascontiguousarray(oP.astype(BF))
        in_maps.append(m)
    return in_maps


def kernel(**inputs):
    nc = _get_nc()
    in_maps = make_in_maps(inputs)
    res = run_bass_kernel_spmd(nc, in_maps, list(range(NCORES)))
    outs = [res.results[c]["out"].transpose(1, 2, 0) for c in range(NCORES)]  # -> [N, BL, D]
    return np.concatenate(outs, axis=1)


# revision 37
# speedup vs baseline: 1.0233x; 1.0233x over previous
"""Trainium2 Bass kernel for the interaction-network GNN (nn_Physics_7370163880185).

Reference computation (per batch element b, objects i=0..15, D=256):
  trans  = MLP_t(objs)                              # 256->512->512->256, relu x3
  pair(i,j) = concat(objs_i, objs_j)                # [512]
  inter  = MLP_i(pair)                              # 512->512->512->256, relu x3
  out    = trans + sum_{j != i} inter(i,j) + objs

Sharding: data-parallel over batch B=512 across 8 cores (64 per core).

Kernel strategy (per core):
  * Feature-on-partition layout: activations are [feat, rows] with
    rows = (n, b) flattened; matmul(out, lhsT=W[k,m], rhs=xT[k, rows])
    chains layers without transposes.
  * Interaction layer 1 is split: concat(a,b) @ iW1 = a @ iW1[:256] + b @ iW1[256:],
    so U = objs @ iW1[:256] + ib1 and V = objs @ iW1[256:] are computed once on
    N*B rows; h1(i,j) = relu(U_i + V_j) is a broadcast add.
  * All matmuls run in bf16 (inputs rounded host-side; fp32 PSUM accumulate).
  * Phase 2 iterates over rotations s=1..15: rotation s computes
    inter(i, (i+s) mod 16) for all i at once in (i, b) column order, so the
    masked diagonal is never computed, h1 = relu(U + roll(V, s)) is two
    dense shifted adds (no broadcast), and the j-sum is a running
    accumulation acc += relu(L3 + ib3) (no masking, no tree reduce).
  * Engine balance: DVE does the h1 adds (bf16 2x) + in-place relu (bf16
    4x); the scalar engine drains all PSUM outputs; gpsimd owns the acc
    accumulation; h1 for s=1,2 is emitted between phase-1 layers so the
    DVE reaches it before the t-MLP finishes on the PE.
  * Weights/biases are packed host-side into per-tensor [128, nk*fout]
    layouts; DMAs stay per-k-tile (parallel rings beat one big serial DMA).
"""

import numpy as np

import concourse.bass as bass
import concourse.mybir as mybir
import concourse.tile as tile
from concourse import bacc
from concourse.bass_utils import run_bass_kernel_spmd

N = 16
B = 512
D = 256
NCORES = 8
BL = B // NCORES          # 64 batch rows per core
ROWS = N * BL             # 1024 (n, b) rows per core
PT = 128                  # partition tile
NT = 512                  # matmul free-dim tile (one PSUM bank of fp32)

F32 = mybir.dt.float32
BF16 = mybir.dt.bfloat16
RELU = mybir.ActivationFunctionType.Relu
IDENT = mybir.ActivationFunctionType.Identity
ADD = mybir.AluOpType.add
MAX = mybir.AluOpType.max

# weight tensors, packed [128, nk*fout] host-side (k-tiles side by side)
WEIGHT_SHAPES = {
    "tW1": (256, 512), "tW2": (512, 512), "tW3": (512, 256),
    "iW1": (256, 512), "iW1b": (256, 512),
    "iW2": (512, 512), "iW3": (512, 256),
}
# biases packed into one [128, 20] tensor, in this column order
BIAS_COLS = {"tb1": (0, 4), "tb2": (4, 8), "tb3": (8, 10),
             "ib1": (10, 14), "ib2": (14, 18), "ib3": (18, 20)}


def _build_body(nc, tc, prm, ctx):
    cpool = ctx.enter_context(tc.tile_pool(name="const", bufs=1))
    wpool = ctx.enter_context(tc.tile_pool(name="work", bufs=2))
    ppool = ctx.enter_context(tc.tile_pool(name="psum", bufs=2, space="PSUM"))

    # ---- inputs: packed SBUF layout, per-k-tile DMAs (parallel rings),
    # in first-use order ---------------------------------------------------
    biases = cpool.tile([PT, 20], F32, tag="biases", bufs=1, name="biases")
    nc.sync.dma_start(out=biases, in_=prm["biases"][:, :])

    objsb = cpool.tile([PT, 2 * ROWS], BF16, tag="objsb", bufs=1, name="objsb")
    for p in range(2):
        nc.sync.dma_start(out=objsb[:, p * ROWS:(p + 1) * ROWS],
                          in_=prm["objsb"][:, p * ROWS:(p + 1) * ROWS])
    objsT_b = [objsb[:, p * ROWS:(p + 1) * ROWS] for p in range(2)]

    w_sb = {}

    def load_weights(*names):
        for wname in names:
            fin, fout = WEIGHT_SHAPES[wname]
            nk = fin // PT
            t = cpool.tile([PT, nk * fout], BF16, tag=wname, bufs=1, name=wname)
            for k in range(nk):
                nc.sync.dma_start(out=t[:, k * fout:(k + 1) * fout],
                                  in_=prm[wname][:, k * fout:(k + 1) * fout])
            w_sb[wname] = t

    # only what phase 1's start needs; the rest is DMA'd mid-phase-1 so the
    # first matmuls don't sit behind the whole input queue
    load_weights("iW1", "iW1b", "tW1", "tW2", "tW3", "iW2", "iW3")

    def wslice(wname, k, m):
        fout = WEIGHT_SHAPES[wname][1]
        return w_sb[wname][:, k * fout + m * PT:k * fout + (m + 1) * PT]

    def bias(bname, m):
        lo, hi = BIAS_COLS[bname]
        assert lo + m < hi
        return biases[:, lo + m:lo + m + 1]

    objsf = cpool.tile([PT, 2 * ROWS], F32, tag="objsf", bufs=1, name="objsf")
    for p in range(2):
        nc.sync.dma_start(out=objsf[:, p * ROWS:(p + 1) * ROWS],
                          in_=prm["objs"][:, p * ROWS:(p + 1) * ROWS])
    objsT_f = [objsf[:, p * ROWS:(p + 1) * ROWS] for p in range(2)]

    # ---- generic dense layer over the full ROWS ------------------------
    # Pairs of column-chunks share each 128-col weight tile so LDWEIGHTS
    # amortizes over 2 matmuls, and consecutive MMs hit different banks.
    def layer(wname, rhs, drain):
        fin, fout = WEIGHT_SHAPES[wname]
        nk = fin // PT
        nm = fout // PT
        ncols = rhs[0].shape[-1]
        mc = [(m, c) for m in range(nm) for c in range(ncols // NT)]
        for g in range(0, len(mc), 2):
            grp = mc[g:g + 2]
            pss = [ppool.tile([PT, NT], F32, tag="ps", bufs=8,
                              name=f"ps_{wname}_{m}_{c}") for (m, c) in grp]
            for k in range(nk):
                for (m, c), ps in zip(grp, pss):
                    nc.tensor.matmul(
                        ps, wslice(wname, k, m),
                        rhs[k][:, c * NT:(c + 1) * NT],
                        start=(k == 0), stop=(k == nk - 1))
            for (m, c), ps in zip(grp, pss):
                drain(m, c, ps)

    def persist(tag, n_tiles, dt=BF16, cols=ROWS):
        return [cpool.tile([PT, cols], dt, tag=f"{tag}_{m}", bufs=1, name=f"{tag}_{m}")
                for m in range(n_tiles)]

    # ---- phase 1: U, V, trans MLP (rows = (n, b), 1024) -----------------
    U = persist("U", 4)       # bf16(objs @ iW1[:256] + ib1)
    V = persist("V", 4)       # bf16(objs @ iW1[256:])
    t1r = persist("t1r", 4)
    t2r = persist("t2r", 4)
    t3 = persist("t3", 2, F32)   # becomes S = t3 + objs after in-place add

    # h1 for rotation s: h1 = relu(U + roll(V, s)) — two dense shifted adds
    # plus an in-place relu, all on DVE (bf16 2x / 4x modes).
    def make_h1(s):
        sp = (N - s) * BL     # split point: i < N-s reads V at +s*BL
        h1r = [wpool.tile([PT, ROWS], BF16, tag=f"h1r_{p}", bufs=3,
                          name=f"h1r_{s}_{p}") for p in range(4)]
        for p in range(4):
            nc.vector.tensor_add(h1r[p][:, 0:sp], U[p][:, 0:sp],
                                 V[p][:, s * BL:ROWS])
            nc.vector.tensor_add(h1r[p][:, sp:ROWS], U[p][:, sp:ROWS],
                                 V[p][:, 0:s * BL])
            nc.vector.tensor_scalar(h1r[p], h1r[p], 0.0, None, MAX)
        return h1r

    # Phase-1 drains stay off the DVE (scalar + gpsimd) so the DVE queue
    # reaches the first rotations' h1 work while the PE runs the t-MLP.
    h1_pre = {}
    layer("iW1", objsT_b,
          lambda m, c, ps: nc.scalar.activation(
              U[m][:, c * NT:(c + 1) * NT], ps, IDENT, bias=bias("ib1", m)))
    layer("iW1b", objsT_b,
          lambda m, c, ps: nc.vector.tensor_copy(V[m][:, c * NT:(c + 1) * NT], ps))
    h1_pre[1] = make_h1(1)
    layer("tW1", objsT_b,
          lambda m, c, ps: nc.scalar.activation(
              t1r[m][:, c * NT:(c + 1) * NT], ps, RELU, bias=bias("tb1", m)))
    h1_pre[2] = make_h1(2)
    layer("tW2", t1r,
          lambda m, c, ps: nc.scalar.activation(
              t2r[m][:, c * NT:(c + 1) * NT], ps, RELU, bias=bias("tb2", m)))
    layer("tW3", t2r,
          lambda m, c, ps: nc.scalar.activation(
              t3[m][:, c * NT:(c + 1) * NT], ps, RELU, bias=bias("tb3", m)))
    # S = t3 + objs  (in place, gpsimd: DVE is on the h1 chain)
    for p in range(2):
        nc.gpsimd.tensor_add(t3[p], t3[p], objsT_f[p])

    # ---- phase 2: rotation loop ----------------------------------------
    # Rotation s pairs every i with j = (i+s) mod 16 at once, skipping the
    # masked diagonal entirely. Column order everywhere is (i, b), matching
    # U, t3 and the output, so the j-sum becomes a running accumulation
    # acc += relu(L3 + ib3) with no masking or tree.
    acc = [cpool.tile([PT, ROWS], F32, tag=f"acc_{p}", bufs=1, name=f"acc_{p}")
           for p in range(2)]
    for s in range(1, N):
        h1r = h1_pre.pop(s) if s in h1_pre else make_h1(s)

        # L2: groups of (m, ih-pair): 2 banks per group, each 128-col weight
        # feeds 2 consecutive MMs (ih inner), 4 groups pipeline in 8 banks
        xT2 = [[wpool.tile([PT, NT], BF16, tag=f"xT2_{ih}_{m}", bufs=2,
                           name=f"xT2_{s}_{ih}_{m}") for m in range(4)]
               for ih in range(2)]
        for m in range(4):
            pss = [ppool.tile([PT, NT], F32, tag="ps", bufs=8,
                              name=f"psL2_{s}_{ih}_{m}") for ih in range(2)]
            for k in range(4):
                for ih in range(2):
                    nc.tensor.matmul(pss[ih], wslice("iW2", k, m),
                                     h1r[k][:, ih * NT:(ih + 1) * NT],
                                     start=(k == 0), stop=(k == 3))
            for ih in range(2):
                nc.scalar.activation(xT2[ih][m], pss[ih], RELU,
                                     bias=bias("ib2", m))
        # L3 + accumulate: s==1 drains straight into acc; later rotations
        # drain to a temp (scalar) and gpsimd adds it into acc.
        for m in range(2):
            pss3 = [ppool.tile([PT, NT], F32, tag="ps", bufs=8,
                               name=f"psL3_{s}_{ih}_{m}") for ih in range(2)]
            for k in range(4):
                for ih in range(2):
                    nc.tensor.matmul(pss3[ih], wslice("iW3", k, m),
                                     xT2[ih][k], start=(k == 0), stop=(k == 3))
            for ih in range(2):
                cs = slice(ih * NT, (ih + 1) * NT)
                if s == 1:
                    nc.scalar.activation(acc[m][:, cs], pss3[ih], RELU,
                                         bias=bias("ib3", m))
                elif s < N - 1:
                    tmp = wpool.tile([PT, NT], F32, tag=f"tmp_{ih}_{m}",
                                     bufs=2, name=f"tmp_{s}_{ih}_{m}")
                    nc.scalar.activation(tmp, pss3[ih], RELU,
                                         bias=bias("ib3", m))
                    nc.gpsimd.tensor_add(acc[m][:, cs], acc[m][:, cs], tmp)
                else:
                    # last rotation: finish this (m, ih) quarter end-to-end
                    # (acc-add + residual + store) so the tail pipelines
                    tmp = wpool.tile([PT, NT], F32, tag=f"tmp_{ih}_{m}",
                                     bufs=2, name=f"tmp_{s}_{ih}_{m}")
                    nc.scalar.activation(tmp, pss3[ih], RELU,
                                         bias=bias("ib3", m))
                    nc.vector.tensor_add(tmp, acc[m][:, cs], tmp)
                    osb = wpool.tile([PT, NT], F32, tag=f"osb_{ih}_{m}",
                                     bufs=1, name=f"osb_{ih}_{m}")
                    nc.vector.tensor_add(osb, tmp, t3[m][:, cs])
                    nc.sync.dma_start(
                        out=prm["out"].rearrange(
                            "(h p) n b -> p h n b", h=2)[:, m, ih * 8:(ih + 1) * 8],
                        in_=osb.rearrange("p (n b) -> p n b", n=8))


def build_nc(loop_iters=None):
    """loop_iters: if set, wrap the whole body in a hardware For_i loop that
    repeats it that many times (used only for timing measurements)."""
    nc = bacc.Bacc("TRN2", target_bir_lowering=False, debug=False)
    prm = {}
    prm["objs"] = nc.declare_dram_parameter("objs", [PT, 2 * ROWS], F32, isOutput=False)
    prm["objsb"] = nc.declare_dram_parameter("objsb", [PT, 2 * ROWS], BF16, isOutput=False)
    for wname, (fin, fout) in WEIGHT_SHAPES.items():
        nk = fin // PT
        prm[wname] = nc.declare_dram_parameter(wname, [PT, nk * fout], BF16, isOutput=False)
    prm["biases"] = nc.declare_dram_parameter("biases", [PT, 20], F32, isOutput=False)
    prm["out"] = nc.declare_dram_parameter("out", [D, N, BL], F32, isOutput=True)
    from contextlib import ExitStack
    with tile.TileContext(nc) as tc:
        if loop_iters is None:
            with ExitStack() as ctx:
                _build_body(nc, tc, prm, ctx)
        else:
            with tc.For_i(0, loop_iters, 1):
                with ExitStack() as ctx:
                    _build_body(nc, tc, prm, ctx)
    nc.compile()
    return nc


_CACHE = {}


def _get_nc():
    if "nc" not in _CACHE:
        _CACHE["nc"] = build_nc()
    return _CACHE["nc"]


def _pack_ktiles(w):
    """[fin, fout] -> [128, nk*fout] with k-tiles side by side."""
    fin, fout = w.shape
    nk = fin // PT
    return np.ascontiguousarray(
        w.reshape(nk, PT, fout).transpose(1, 0, 2).reshape(PT, nk * fout))


def make_in_maps(inputs):
    import ml_dtypes
    BF = ml_dtypes.bfloat16
    shared = {}
    for name in ("tW1", "tW2", "tW3", "iW2", "iW3"):
        shared[name] = _pack_ktiles(
            np.asarray(inputs[name], dtype=np.float32)).astype(BF)
    iW1 = np.asarray(inputs["iW1"], dtype=np.float32)
    shared["iW1"] = _pack_ktiles(iW1[:D]).astype(BF)
    shared["iW1b"] = _pack_ktiles(iW1[D:]).astype(BF)
    bcols = []
    for bname in BIAS_COLS:
        b = np.asarray(inputs[bname], dtype=np.float32)
        bcols.append(b.reshape(-1, PT).T)       # [128, nb]
    shared["biases"] = np.ascontiguousarray(np.concatenate(bcols, axis=1))
    objs = np.asarray(inputs["objs"], dtype=np.float32)
    in_maps = []
    for c in range(NCORES):
        m = dict(shared)
        sl = objs[:, c * BL:(c + 1) * BL, :]            # [N, BL, D]
        oT = sl.transpose(2, 0, 1).reshape(D, ROWS)     # [256, 1024]
        oP = np.ascontiguousarray(
            oT.reshape(2, PT, ROWS).transpose(1, 0, 2).reshape(PT, 2 * ROWS))
        m["objs"] = oP
        m["objsb"] = np.ascontiguousarray(oP.astype(BF))
        in_maps.append(m)
    return in_maps


def kernel(**inputs):
    nc = _get_nc()
    in_maps = make_in_maps(inputs)
    res = run_bass_kernel_spmd(nc, in_maps, list(range(NCORES)))
    outs = [res.results[c]["out"].transpose(1, 2, 0) for c in range(NCORES)]  # -> [N, BL, D]
    return np.concatenate(outs, axis=1)


# revision 38
# speedup vs baseline: 1.0611x; 1.0369x over previous
"""Trainium2 Bass kernel for the interaction-network GNN (nn_Physics_7370163880185).

Reference computation (per batch element b, objects i=0..15, D=256):
  trans  = MLP_t(objs)                              # 256->512->512->256, relu x3
  pair(i,j) = concat(objs_i, objs_j)                # [512]
  inter  = MLP_i(pair)                              # 512->512->512->256, relu x3
  out    = trans + sum_{j != i} inter(i,j) + objs

Sharding: data-parallel over batch B=512 across 8 cores (64 per core).

Kernel strategy (per core):
  * Feature-on-partition layout: activations are [feat, rows] with
    rows = (n, b) flattened; matmul(out, lhsT=W[k,m], rhs=xT[k, rows])
    chains layers without transposes.
  * Interaction layer 1 is split: concat(a,b) @ iW1 = a @ iW1[:256] + b @ iW1[256:],
    so U = objs @ iW1[:256] + ib1 and V = objs @ iW1[256:] are computed once on
    N*B rows; h1(i,j) = relu(U_i + V_j) is a broadcast add.
  * All matmuls run in bf16 (inputs rounded host-side; fp32 PSUM accumulate).
  * Phase 2 iterates over rotations s=1..15: rotation s computes
    inter(i, (i+s) mod 16) for all i at once in (i, b) column order, so the
    masked diagonal is never computed, h1 = relu(U + roll(V, s)) is two
    dense shifted adds (no broadcast), and the j-sum is a running
    accumulation acc += relu(L3 + ib3) (no masking, no tree reduce).
  * Engine balance: DVE does the h1 adds (bf16 2x) + in-place relu (bf16
    4x); the scalar engine drains all PSUM outputs; gpsimd owns the acc
    accumulation; h1 for s=1,2 is emitted between phase-1 layers so the
    DVE reaches it before the t-MLP finishes on the PE.
  * Weights/biases are packed host-side into per-tensor [128, nk*fout]
    layouts; DMAs stay per-k-tile (parallel rings beat one big serial DMA).
"""

import numpy as np

import concourse.bass as bass
import concourse.mybir as mybir
import concourse.tile as tile
from concourse import bacc
from concourse.bass_utils import run_bass_kernel_spmd

N = 16
B = 512
D = 256
NCORES = 8
BL = B // NCORES          # 64 batch rows per core
ROWS = N * BL             # 1024 (n, b) rows per core
PT = 128                  # partition tile
NT = 512                  # matmul free-dim tile (one PSUM bank of fp32)

F32 = mybir.dt.float32
BF16 = mybir.dt.bfloat16
RELU = mybir.ActivationFunctionType.Relu
IDENT = mybir.ActivationFunctionType.Identity
ADD = mybir.AluOpType.add
MAX = mybir.AluOpType.max

# weight tensors, packed [128, nk*fout] host-side (k-tiles side by side)
WEIGHT_SHAPES = {
    "tW1": (256, 512), "tW2": (512, 512), "tW3": (512, 256),
    "iW1": (256, 512), "iW1b": (256, 512),
    "iW2": (512, 512), "iW3": (512, 256),
}
# biases packed into one [128, 20] tensor, in this column order
BIAS_COLS = {"tb1": (0, 4), "tb2": (4, 8), "tb3": (8, 10),
             "ib1": (10, 14), "ib2": (14, 18), "ib3": (18, 20)}


def _build_body(nc, tc, prm, ctx):
    cpool = ctx.enter_context(tc.tile_pool(name="const", bufs=1))
    wpool = ctx.enter_context(tc.tile_pool(name="work", bufs=2))
    ppool = ctx.enter_context(tc.tile_pool(name="psum", bufs=2, space="PSUM"))

    # ---- inputs: packed SBUF layout, per-k-tile DMAs (parallel rings),
    # in first-use order ---------------------------------------------------
    biases = cpool.tile([PT, 20], F32, tag="biases", bufs=1, name="biases")
    nc.sync.dma_start(out=biases, in_=prm["biases"][:, :])

    objsb = cpool.tile([PT, 2 * ROWS], BF16, tag="objsb", bufs=1, name="objsb")
    for p in range(2):
        nc.sync.dma_start(out=objsb[:, p * ROWS:(p + 1) * ROWS],
                          in_=prm["objsb"][:, p * ROWS:(p + 1) * ROWS])
    objsT_b = [objsb[:, p * ROWS:(p + 1) * ROWS] for p in range(2)]

    w_sb = {}

    def load_weights(*names):
        for wname in names:
            fin, fout = WEIGHT_SHAPES[wname]
            nk = fin // PT
            t = cpool.tile([PT, nk * fout], BF16, tag=wname, bufs=1, name=wname)
            for k in range(nk):
                nc.sync.dma_start(out=t[:, k * fout:(k + 1) * fout],
                                  in_=prm[wname][:, k * fout:(k + 1) * fout])
            w_sb[wname] = t

    # only what phase 1's start needs; the rest is DMA'd mid-phase-1 so the
    # first matmuls don't sit behind the whole input queue
    load_weights("iW1", "iW1b", "tW1", "tW2", "tW3", "iW2", "iW3")

    def wslice(wname, k, m):
        fout = WEIGHT_SHAPES[wname][1]
        return w_sb[wname][:, k * fout + m * PT:k * fout + (m + 1) * PT]

    def bias(bname, m):
        lo, hi = BIAS_COLS[bname]
        assert lo + m < hi
        return biases[:, lo + m:lo + m + 1]

    objsf = cpool.tile([PT, 2 * ROWS], F32, tag="objsf", bufs=1, name="objsf")
    for p in range(2):
        nc.sync.dma_start(out=objsf[:, p * ROWS:(p + 1) * ROWS],
                          in_=prm["objs"][:, p * ROWS:(p + 1) * ROWS])
    objsT_f = [objsf[:, p * ROWS:(p + 1) * ROWS] for p in range(2)]

    # ---- generic dense layer over the full ROWS ------------------------
    # Pairs of column-chunks share each 128-col weight tile so LDWEIGHTS
    # amortizes over 2 matmuls, and consecutive MMs hit different banks.
    def layer(wname, rhs, drain):
        fin, fout = WEIGHT_SHAPES[wname]
        nk = fin // PT
        nm = fout // PT
        ncols = rhs[0].shape[-1]
        mc = [(m, c) for m in range(nm) for c in range(ncols // NT)]
        for g in range(0, len(mc), 2):
            grp = mc[g:g + 2]
            pss = [ppool.tile([PT, NT], F32, tag="ps", bufs=8,
                              name=f"ps_{wname}_{m}_{c}") for (m, c) in grp]
            for k in range(nk):
                for (m, c), ps in zip(grp, pss):
                    nc.tensor.matmul(
                        ps, wslice(wname, k, m),
                        rhs[k][:, c * NT:(c + 1) * NT],
                        start=(k == 0), stop=(k == nk - 1))
            for (m, c), ps in zip(grp, pss):
                drain(m, c, ps)

    def persist(tag, n_tiles, dt=BF16, cols=ROWS):
        return [cpool.tile([PT, cols], dt, tag=f"{tag}_{m}", bufs=1, name=f"{tag}_{m}")
                for m in range(n_tiles)]

    # ---- phase 1: U, V, trans MLP (rows = (n, b), 1024) -----------------
    U = persist("U", 4)       # bf16(objs @ iW1[:256] + ib1)
    V = persist("V", 4)       # bf16(objs @ iW1[256:])
    t1r = persist("t1r", 4)
    t2r = persist("t2r", 4)
    t3 = persist("t3", 2, F32)   # becomes S = t3 + objs after in-place add

    # h1 for rotation s: h1 = relu(U + roll(V, s)) — two dense shifted adds
    # plus an in-place relu, all on DVE (bf16 2x / 4x modes).
    def make_h1(s):
        sp = (N - s) * BL     # split point: i < N-s reads V at +s*BL
        h1r = [wpool.tile([PT, ROWS], BF16, tag=f"h1r_{p}", bufs=3,
                          name=f"h1r_{s}_{p}") for p in range(4)]
        for p in range(4):
            nc.vector.tensor_add(h1r[p][:, 0:sp], U[p][:, 0:sp],
                                 V[p][:, s * BL:ROWS])
            nc.vector.tensor_add(h1r[p][:, sp:ROWS], U[p][:, sp:ROWS],
                                 V[p][:, 0:s * BL])
            nc.vector.tensor_scalar(h1r[p], h1r[p], 0.0, None, MAX)
        return h1r

    # Phase-1 drains stay off the DVE (scalar + gpsimd) so the DVE queue
    # reaches the first rotations' h1 work while the PE runs the t-MLP.
    h1_pre = {}
    layer("iW1", objsT_b,
          lambda m, c, ps: nc.scalar.activation(
              U[m][:, c * NT:(c + 1) * NT], ps, IDENT, bias=bias("ib1", m)))
    layer("iW1b", objsT_b,
          lambda m, c, ps: nc.vector.tensor_copy(V[m][:, c * NT:(c + 1) * NT], ps))
    h1_pre[1] = make_h1(1)
    layer("tW1", objsT_b,
          lambda m, c, ps: nc.scalar.activation(
              t1r[m][:, c * NT:(c + 1) * NT], ps, RELU, bias=bias("tb1", m)))
    h1_pre[2] = make_h1(2)
    layer("tW2", t1r,
          lambda m, c, ps: nc.scalar.activation(
              t2r[m][:, c * NT:(c + 1) * NT], ps, RELU, bias=bias("tb2", m)))
    layer("tW3", t2r,
          lambda m, c, ps: nc.scalar.activation(
              t3[m][:, c * NT:(c + 1) * NT], ps, RELU, bias=bias("tb3", m)))
    # S = t3 + objs  (in place, gpsimd: DVE is on the h1 chain)
    for p in range(2):
        nc.gpsimd.tensor_add(t3[p], t3[p], objsT_f[p])

    # ---- phase 2: rotation loop ----------------------------------------
    # Rotation s pairs every i with j = (i+s) mod 16 at once, skipping the
    # masked diagonal entirely. Column order everywhere is (i, b), matching
    # U, t3 and the output, so the j-sum becomes a running accumulation
    # acc += relu(L3 + ib3) with no masking or tree.
    acc = [cpool.tile([PT, ROWS], F32, tag=f"acc_{p}", bufs=1, name=f"acc_{p}")
           for p in range(2)]
    osb_pre = [cpool.tile([PT, ROWS], F32, tag=f"osb_pre_{p}", bufs=1,
                          name=f"osb_pre_{p}") for p in range(2)]
    for s in range(1, N):
        if s == N - 1:
            # acc holds s=1..14; fold in the residual early so the last
            # rotation's tail is just drain -> add -> store per quarter
            for p in range(2):
                nc.gpsimd.tensor_add(osb_pre[p], acc[p], t3[p])
        h1r = h1_pre.pop(s) if s in h1_pre else make_h1(s)

        # L2: groups of (m, ih-pair): 2 banks per group, each 128-col weight
        # feeds 2 consecutive MMs (ih inner), 4 groups pipeline in 8 banks
        xT2 = [[wpool.tile([PT, NT], BF16, tag=f"xT2_{ih}_{m}", bufs=2,
                           name=f"xT2_{s}_{ih}_{m}") for m in range(4)]
               for ih in range(2)]
        for m in range(4):
            pss = [ppool.tile([PT, NT], F32, tag="ps", bufs=8,
                              name=f"psL2_{s}_{ih}_{m}") for ih in range(2)]
            for k in range(4):
                for ih in range(2):
                    nc.tensor.matmul(pss[ih], wslice("iW2", k, m),
                                     h1r[k][:, ih * NT:(ih + 1) * NT],
                                     start=(k == 0), stop=(k == 3))
            for ih in range(2):
                nc.scalar.activation(xT2[ih][m], pss[ih], RELU,
                                     bias=bias("ib2", m))
        # L3 + accumulate: s==1 drains straight into acc; later rotations
        # drain to a temp (scalar) and gpsimd adds it into acc.
        for m in range(2):
            pss3 = [ppool.tile([PT, NT], F32, tag="ps", bufs=8,
                               name=f"psL3_{s}_{ih}_{m}") for ih in range(2)]
            for k in range(4):
                for ih in range(2):
                    nc.tensor.matmul(pss3[ih], wslice("iW3", k, m),
                                     xT2[ih][k], start=(k == 0), stop=(k == 3))
            for ih in range(2):
                cs = slice(ih * NT, (ih + 1) * NT)
                if s == 1:
                    nc.scalar.activation(acc[m][:, cs], pss3[ih], RELU,
                                         bias=bias("ib3", m))
                elif s < N - 1:
                    tmp = wpool.tile([PT, NT], F32, tag=f"tmp_{ih}_{m}",
                                     bufs=2, name=f"tmp_{s}_{ih}_{m}")
                    nc.scalar.activation(tmp, pss3[ih], RELU,
                                         bias=bias("ib3", m))
                    nc.gpsimd.tensor_add(acc[m][:, cs], acc[m][:, cs], tmp)
                else:
                    # last rotation: finish this (m, ih) quarter end-to-end
                    # (drain + one add against osb_pre + store)
                    tmp = wpool.tile([PT, NT], F32, tag=f"tmp_{ih}_{m}",
                                     bufs=2, name=f"tmp_{s}_{ih}_{m}")
                    nc.scalar.activation(tmp, pss3[ih], RELU,
                                         bias=bias("ib3", m))
                    osb = wpool.tile([PT, NT], F32, tag=f"osb_{ih}_{m}",
                                     bufs=1, name=f"osb_{ih}_{m}")
                    nc.vector.tensor_add(osb, tmp, osb_pre[m][:, cs])
                    nc.sync.dma_start(
                        out=prm["out"].rearrange(
                            "(h p) n b -> p h n b", h=2)[:, m, ih * 8:(ih + 1) * 8],
                        in_=osb.rearrange("p (n b) -> p n b", n=8))


def build_nc(loop_iters=None):
    """loop_iters: if set, wrap the whole body in a hardware For_i loop that
    repeats it that many times (used only for timing measurements)."""
    nc = bacc.Bacc("TRN2", target_bir_lowering=False, debug=False)
    prm = {}
    prm["objs"] = nc.declare_dram_parameter("objs", [PT, 2 * ROWS], F32, isOutput=False)
    prm["objsb"] = nc.declare_dram_parameter("objsb", [PT, 2 * ROWS], BF16, isOutput=False)
    for wname, (fin, fout) in WEIGHT_SHAPES.items():
        nk = fin // PT
        prm[wname] = nc.declare_dram_parameter(wname, [PT, nk * fout], BF16, isOutput=False)
    prm["biases"] = nc.declare_dram_parameter("biases", [PT, 20], F32, isOutput=False)
    prm["out"] = nc.declare_dram_parameter("out", [D, N, BL], F32, isOutput=True)
    from contextlib import ExitStack
    with tile.TileContext(nc) as tc:
        if loop_iters is None:
            with ExitStack() as ctx:
                _build_body(nc, tc, prm, ctx)
        else:
            with tc.For_i(0, loop_iters, 1):
                with ExitStack() as ctx:
                    _build_body(nc, tc, prm, ctx)
    nc.compile()
    return nc


_CACHE = {}


def _get_nc():
    if "nc" not in _CACHE:
        _CACHE["nc"] = build_nc()
    return _CACHE["nc"]


def _pack_ktiles(w):
    """[fin, fout] -> [128, nk*fout] with k-tiles side by side."""
    fin, fout = w.shape
    nk = fin // PT
    return np.ascontiguousarray(
        w.reshape(nk, PT, fout).transpose(1, 0, 2).reshape(PT, nk * fout))


def make_in_maps(inputs):
    import ml_dtypes
    BF = ml_dtypes.bfloat16
    shared = {}
    for name in ("tW1", "tW2", "tW3", "iW2", "iW3"):
        shared[name] = _pack_ktiles(
            np.asarray(inputs[name], dtype=np.float32)).astype(BF)
    iW1 = np.asarray(inputs["iW1"], dtype=np.float32)
    shared["iW1"] = _pack_ktiles(iW1[:D]).astype(BF)
    shared["iW1b"] = _pack_ktiles(iW1[D:]).astype(BF)
    bcols = []
    for bname in BIAS_COLS:
        b = np.asarray(inputs[bname], dtype=np.float32)
        bcols.append(b.reshape(-1, PT).T)       # [128, nb]
    shared["biases"] = np.ascontiguousarray(np.concatenate(bcols, axis=1))
    objs = np.asarray(inputs["objs"], dtype=np.float32)
    in_maps = []
    for c in range(NCORES):
        m = dict(shared)
        sl = objs[:, c * BL:(c + 1) * BL, :]            # [N, BL, D]
        oT = sl.transpose(2, 0, 1).reshape(D, ROWS)     # [256, 1024]
        oP = np.ascontiguousarray(
            oT.reshape(2, PT, ROWS).transpose(1, 0, 2).reshape(PT, 2 * ROWS))
        m["objs"] = oP
        m["objsb"] = np.ascontiguousarray(oP.astype(BF))
        in_maps.append(m)
    return in_maps


def kernel(**inputs):
    nc = _get_nc()
    in_maps = make_in_maps(inputs)
    res = run_bass_kernel_spmd(nc, in_maps, list(range(NCORES)))
    outs = [res.results[c]["out"].transpose(1, 2, 0) for c in range(NCORES)]  # -> [N, BL, D]
    return np.concatenate(outs, axis=1)
